# revision 5
# baseline (speedup 1.0000x reference)
"""Self-contained Trainium2 kernel for nn_Net_69183333204554 (GNN message
passing: EdgePooling -> 2x ChebConv+BN+ReLU -> global max/mean pool -> MLP).

Strategy (8 NeuronCores, SPMD):
  K12: Z = x @ [c1_w0*a1 | c1_w1*a1]  -- node-sharded dense matmul (fp32r PE),
       the dominant memory/compute pass over x (134 MB).
  host: edge scores (fp32, validated to reproduce the reference's greedy
       matching exactly), greedy maximal matching, cluster/coarsen/dedup,
       index-plumbing, routing of gathered rows between phases.
  K3:  ChebConv1 message passing (indirect row gathers + indicator-matmul
       segment-sum on PE) + ReLU + ChebConv2 dense matmuls.
  K4:  ChebConv2 message passing + ReLU + global max/sum pooling on device.
  host: tiny readout MLP ([8,1024] -> [8,4]/[8,256]).

All shapes hardcoded for the fixed problem instance (N=32768, E=131072, B=8).
"""
import numpy as np
import ml_dtypes
from contextlib import ExitStack

import concourse.bass as bass
import concourse.tile as tile
from concourse import bacc, mybir
from concourse.bass_utils import run_bass_kernel_spmd
from concourse.masks import make_identity

F32 = mybir.dt.float32
F32R = mybir.dt.float32r
BF16 = mybir.dt.bfloat16
I32 = mybir.dt.int32

NCORE = 8
N = 32768
E = 131072
B = 8
F = 1025
NPC = N // NCORE          # 4096 nodes per core in K12
KCH = 8                   # full 128-rows K chunks in F (1025 = 8*128 + 1)
NVP = 2304                # padded valid rows per core (18 tiles of 128)
NWIN = NVP // 128         # 18 windows / node tiles per core
CPW = 8                   # max edge chunks (of 128) per 128-dst window
NCH = NWIN * CPW          # 144 chunk slots per core
NU = 10496                # padded unique-source rows per core
EPS = np.float32(1e-5)
NEG = np.float32(-1e30)

# exec times of the last kernel() call, for test harnesses: list of
# (name, exec_time_ns or None)
LAST_EXEC_TIMES = []

_PROGS = {}


def _new_nc():
    return bacc.Bacc("TRN2", target_bir_lowering=False, debug=False,
                     num_devices=NCORE)


# --------------------------------------------------------------------------
# K12: Z[4096, 1024] = xT.T @ wcat      (per core)
# --------------------------------------------------------------------------
def _build_k12():
    nc = _new_nc()
    xT = nc.dram_tensor("xT", [F, NPC], F32R, kind="ExternalInput").ap()
    wcat = nc.dram_tensor("wcat", [F, 1024], F32R, kind="ExternalInput").ap()
    z = nc.dram_tensor("z", [NPC, 1024], F32, kind="ExternalOutput").ap()

    with tile.TileContext(nc) as tc, ExitStack() as ctx:
        wpool = ctx.enter_context(tc.tile_pool(name="w", bufs=1))
        xpool = ctx.enter_context(tc.tile_pool(name="x", bufs=2))
        zpool = ctx.enter_context(tc.tile_pool(name="z", bufs=4))
        psum = ctx.enter_context(tc.tile_pool(name="ps", bufs=4, space="PSUM"))

        w_sb = []
        for k in range(KCH + 1):
            p = 128 if k < KCH else 1
            t = wpool.tile([p, 1024], F32R, tag=f"w{k}")
            nc.sync.dma_start(t[:], wcat[k * 128:k * 128 + p, :])
            w_sb.append(t)

        NB = NPC // 512  # 8 node blocks of 512
        for jb in range(NB):
            xb = []
            for k in range(KCH + 1):
                p = 128 if k < KCH else 1
                t = xpool.tile([p, 512], F32R, tag=f"x{k}")
                nc.sync.dma_start(t[:], xT[k * 128:k * 128 + p,
                                            jb * 512:(jb + 1) * 512])
                xb.append(t)
            for j4 in range(4):
                js = slice(j4 * 128, (j4 + 1) * 128)
                for h in range(2):
                    hs = slice(h * 512, (h + 1) * 512)
                    pt = psum.tile([128, 512], F32, tag="acc")
                    for k in range(KCH + 1):
                        nc.tensor.matmul(
                            pt[:],
                            lhsT=xb[k][:, js],
                            rhs=w_sb[k][:, hs],
                            start=(k == 0),
                            stop=(k == KCH),
                        )
                    zt = zpool.tile([128, 512], F32, tag="zt")
                    nc.vector.tensor_copy(zt[:], pt[:])
                    nc.sync.dma_start(
                        z[jb * 512 + j4 * 128:jb * 512 + (j4 + 1) * 128, hs],
                        zt[:])
    nc.compile()
    return nc


# --------------------------------------------------------------------------
# K3: message passing 1 + relu + conv2 matmuls  (per core)
# --------------------------------------------------------------------------
def _build_k3():
    nc = _new_nc()
    t1 = nc.dram_tensor("t1", [NVP, 512], F32, kind="ExternalInput").ap()
    y1c = nc.dram_tensor("y1c", [NU, 512], BF16, kind="ExternalInput").ap()
    esrc = nc.dram_tensor("esrc", [128, NCH], I32, kind="ExternalInput").ap()
    edst = nc.dram_tensor("edst", [128, NCH], F32, kind="ExternalInput").ap()
    enrm = nc.dram_tensor("enrm", [128, NCH], F32, kind="ExternalInput").ap()
    w2 = nc.dram_tensor("w2", [513, 1024], F32R, kind="ExternalInput").ap()
    onesr = nc.dram_tensor("onesr", [1, NVP], F32R, kind="ExternalInput").ap()
    zz2 = nc.dram_tensor("zz2", [NVP, 1024], F32, kind="ExternalOutput").ap()

    with tile.TileContext(nc) as tc, ExitStack() as ctx:
        const = ctx.enter_context(tc.tile_pool(name="const", bufs=1))
        gat = ctx.enter_context(tc.tile_pool(name="gat", bufs=6))
        hseq = ctx.enter_context(tc.tile_pool(name="hseq", bufs=3))
        opool = ctx.enter_context(tc.tile_pool(name="o", bufs=4))
        psum = ctx.enter_context(tc.tile_pool(name="ps", bufs=2, space="PSUM"))
        psum2 = ctx.enter_context(tc.tile_pool(name="ps2", bufs=4, space="PSUM"))

        iota_i = const.tile([128, 128], I32)
        nc.gpsimd.iota(iota_i[:], pattern=[[1, 128]], base=0,
                       channel_multiplier=0)
        iota_f = const.tile([128, 128], F32)
        nc.vector.tensor_copy(iota_f[:], iota_i[:])
        ident = const.tile([128, 128], F32)
        make_identity(nc, ident[:])
        identr = const.tile([128, 128], F32R)
        nc.vector.tensor_copy(identr[:], ident[:])

        esrc_sb = const.tile([128, NCH], I32)
        nc.sync.dma_start(esrc_sb[:], esrc[:, :])
        edst_sb = const.tile([128, NCH], F32)
        nc.sync.dma_start(edst_sb[:], edst[:, :])
        enrm_sb = const.tile([128, NCH], F32)
        nc.sync.dma_start(enrm_sb[:], enrm[:, :])

        t1_sb = const.tile([128, NWIN * 512], F32)
        nc.sync.dma_start(
            t1_sb[:].rearrange("p (t f) -> p t f", f=512),
            t1.rearrange("(t p) f -> p t f", p=128))

        onesr_sb = const.tile([1, NVP], F32R)
        nc.sync.dma_start(onesr_sb[:], onesr[:, :])

        w2_sb = []
        for k in range(5):
            p = 128 if k < 4 else 1
            t = const.tile([p, 1024], F32R, tag=f"w2{k}")
            nc.sync.dma_start(t[:], w2[k * 128:k * 128 + p, :])
            w2_sb.append(t)

        h1T = const.tile([128, 4 * NVP], F32R)

        for w in range(NWIN):
            pt = psum.tile([128, 512], F32, tag="tx")
            for cch in range(CPW):
                s = w * CPW + cch
                y_t = gat.tile([128, 512], BF16, tag="y")
                nc.gpsimd.indirect_dma_start(
                    out=y_t[:], out_offset=None, in_=y1c[:, :],
                    in_offset=bass.IndirectOffsetOnAxis(
                        ap=esrc_sb[:, s:s + 1], axis=0))
                ind = gat.tile([128, 128], BF16, tag="ind")
                nc.vector.tensor_scalar(
                    out=ind[:], in0=iota_f[:],
                    scalar1=edst_sb[:, s:s + 1],
                    scalar2=enrm_sb[:, s:s + 1],
                    op0=mybir.AluOpType.is_equal,
                    op1=mybir.AluOpType.mult)
                nc.tensor.matmul(pt[:], lhsT=ind[:], rhs=y_t[:],
                                 start=(cch == 0), stop=(cch == CPW - 1))
            h1_t = hseq.tile([128, 512], F32R, tag="h1")
            nc.vector.tensor_add(h1_t[:], pt[:],
                                 t1_sb[:, w * 512:(w + 1) * 512])
            nc.vector.tensor_scalar_max(h1_t[:], h1_t[:], 0.0)
            for k4 in range(4):
                ptr = psum2.tile([128, 128], F32R, tag="tr")
                nc.tensor.transpose(
                    out=ptr[:],
                    in_=h1_t[:, k4 * 128:(k4 + 1) * 128],
                    identity=identr[:])
                nc.vector.tensor_copy(
                    h1T[:, k4 * NVP + w * 128:k4 * NVP + (w + 1) * 128],
                    ptr[:])

        for j in range(NWIN):
            for h in range(2):
                hs = slice(h * 512, (h + 1) * 512)
                po = psum.tile([128, 512], F32, tag="o")
                for k4 in range(4):
                    nc.tensor.matmul(
                        po[:],
                        lhsT=h1T[:, k4 * NVP + j * 128:
                                 k4 * NVP + (j + 1) * 128],
                        rhs=w2_sb[k4][:, hs],
                        start=(k4 == 0), stop=False)
                nc.tensor.matmul(
                    po[:],
                    lhsT=onesr_sb[:, j * 128:(j + 1) * 128],
                    rhs=w2_sb[4][:, hs],
                    start=False, stop=True)
                ot = opool.tile([128, 512], F32, tag="ot")
                nc.vector.tensor_copy(ot[:], po[:])
                nc.sync.dma_start(zz2[j * 128:(j + 1) * 128, hs], ot[:])
    nc.compile()
    return nc


# --------------------------------------------------------------------------
# K4: message passing 2 + relu + pooling  (per core)
# --------------------------------------------------------------------------
def _build_k4():
    nc = _new_nc()
    t2 = nc.dram_tensor("t2", [NVP, 512], F32, kind="ExternalInput").ap()
    y2c = nc.dram_tensor("y2c", [NU, 512], BF16, kind="ExternalInput").ap()
    esrc = nc.dram_tensor("esrc", [128, NCH], I32, kind="ExternalInput").ap()
    edst = nc.dram_tensor("edst", [128, NCH], F32, kind="ExternalInput").ap()
    enrm = nc.dram_tensor("enrm", [128, NCH], F32, kind="ExternalInput").ap()
    batchg = nc.dram_tensor("batchg", [128, NWIN], F32,
                            kind="ExternalInput").ap()
    hsum = nc.dram_tensor("hsum", [8, 512], F32, kind="ExternalOutput").ap()
    hmaxt = nc.dram_tensor("hmaxt", [128, 4 * NWIN], F32,
                           kind="ExternalOutput").ap()

    with tile.TileContext(nc) as tc, ExitStack() as ctx:
        const = ctx.enter_context(tc.tile_pool(name="const", bufs=1))
        gat = ctx.enter_context(tc.tile_pool(name="gat", bufs=6))
        hseq = ctx.enter_context(tc.tile_pool(name="hseq", bufs=3))
        psum = ctx.enter_context(tc.tile_pool(name="ps", bufs=2, space="PSUM"))
        psum2 = ctx.enter_context(tc.tile_pool(name="ps2", bufs=4, space="PSUM"))
        psmax = ctx.enter_context(tc.tile_pool(name="psm", bufs=1, space="PSUM"))

        iota_i = const.tile([128, 128], I32)
        nc.gpsimd.iota(iota_i[:], pattern=[[1, 128]], base=0,
                       channel_multiplier=0)
        iota_f = const.tile([128, 128], F32)
        nc.vector.tensor_copy(iota_f[:], iota_i[:])
        iota8_f = const.tile([128, 8], F32)
        nc.vector.tensor_copy(iota8_f[:], iota_i[:, :8])
        ident = const.tile([128, 128], F32)
        make_identity(nc, ident[:])
        identr = const.tile([128, 128], F32R)
        nc.vector.tensor_copy(identr[:], ident[:])

        esrc_sb = const.tile([128, NCH], I32)
        nc.sync.dma_start(esrc_sb[:], esrc[:, :])
        edst_sb = const.tile([128, NCH], F32)
        nc.sync.dma_start(edst_sb[:], edst[:, :])
        enrm_sb = const.tile([128, NCH], F32)
        nc.sync.dma_start(enrm_sb[:], enrm[:, :])
        batchg_sb = const.tile([128, NWIN], F32)
        nc.sync.dma_start(batchg_sb[:], batchg[:, :])

        t2_sb = const.tile([128, NWIN * 512], F32)
        nc.sync.dma_start(
            t2_sb[:].rearrange("p (t f) -> p t f", f=512),
            t2.rearrange("(t p) f -> p t f", p=128))

        hmax_sb = const.tile([128, 4 * NWIN], F32)
        ps_hs = psmax.tile([8, 512], F32)

        for w in range(NWIN):
            pt = psum.tile([128, 512], F32, tag="tx")
            for cch in range(CPW):
                s = w * CPW + cch
                y_t = gat.tile([128, 512], BF16, tag="y")
                nc.gpsimd.indirect_dma_start(
                    out=y_t[:], out_offset=None, in_=y2c[:, :],
                    in_offset=bass.IndirectOffsetOnAxis(
                        ap=esrc_sb[:, s:s + 1], axis=0))
                ind = gat.tile([128, 128], BF16, tag="ind")
                nc.vector.tensor_scalar(
                    out=ind[:], in0=iota_f[:],
                    scalar1=edst_sb[:, s:s + 1],
                    scalar2=enrm_sb[:, s:s + 1],
                    op0=mybir.AluOpType.is_equal,
                    op1=mybir.AluOpType.mult)
                nc.tensor.matmul(pt[:], lhsT=ind[:], rhs=y_t[:],
                                 start=(cch == 0), stop=(cch == CPW - 1))
            h2_t = hseq.tile([128, 512], F32R, tag="h2")
            nc.vector.tensor_add(h2_t[:], pt[:],
                                 t2_sb[:, w * 512:(w + 1) * 512])
            nc.vector.tensor_scalar_max(h2_t[:], h2_t[:], 0.0)

            # per-graph sums via batch-indicator matmul (accumulate over all
            # windows into one PSUM tile)
            ind_b = gat.tile([128, 8], F32R, tag="indb")
            nc.vector.tensor_scalar(
                out=ind_b[:], in0=iota8_f[:],
                scalar1=batchg_sb[:, w:w + 1], scalar2=None,
                op0=mybir.AluOpType.is_equal)
            nc.tensor.matmul(ps_hs[:], lhsT=ind_b[:], rhs=h2_t[:],
                             start=(w == 0), stop=(w == NWIN - 1))

            # per-tile channel max via transpose + free-dim reduce
            for k4 in range(4):
                ptr = psum2.tile([128, 128], F32R, tag="tr")
                nc.tensor.transpose(
                    out=ptr[:],
                    in_=h2_t[:, k4 * 128:(k4 + 1) * 128],
                    identity=identr[:])
                nc.vector.tensor_reduce(
                    out=hmax_sb[:, w * 4 + k4:w * 4 + k4 + 1],
                    in_=ptr[:], axis=mybir.AxisListType.X,
                    op=mybir.AluOpType.max)

        hs_sb = const.tile([8, 512], F32)
        nc.vector.tensor_copy(hs_sb[:], ps_hs[:])
        nc.sync.dma_start(hsum[:, :], hs_sb[:])
        nc.sync.dma_start(hmaxt[:, :], hmax_sb[:])
    nc.compile()
    return nc


def _get_prog(name):
    if name not in _PROGS:
        _PROGS[name] = {"k12": _build_k12, "k3": _build_k3,
                        "k4": _build_k4}[name]()
    return _PROGS[name]


def _run(name, in_maps):
    nc = _get_prog(name)
    r = run_bass_kernel_spmd(nc, in_maps, core_ids=list(range(NCORE)))
    LAST_EXEC_TIMES.append((name, r.exec_time_ns))
    return r.results


# --------------------------------------------------------------------------
# host-side pipeline pieces
# --------------------------------------------------------------------------
def _scores(p1, p2, u, v):
    e = p1[u] + p2[v]
    m = np.full(N, -np.inf, dtype=np.float32)
    np.maximum.at(m, v, e)
    ex = np.exp(e - m[v])
    ssum = np.zeros(N, dtype=np.float32)
    np.add.at(ssum, v, ex)
    return (ex / ssum[v] + np.float32(0.5)).astype(np.float32)


def _greedy_match(u, v, order):
    """Sequential greedy matching, replicated via rounds of local-max
    matching (equivalent under the strict total order given by `order`)."""
    rank = np.empty(E, dtype=np.int64)
    rank[order] = np.arange(E)
    used = np.zeros(N, dtype=bool)
    sel = np.zeros(E, dtype=bool)
    INF = np.iinfo(np.int64).max
    cand = np.arange(E)
    while True:
        cand = cand[~(used[u[cand]] | used[v[cand]])]
        if len(cand) == 0:
            break
        r = rank[cand]
        best = np.full(N, INF, dtype=np.int64)
        np.minimum.at(best, u[cand], r)
        np.minimum.at(best, v[cand], r)
        win = (best[u[cand]] == r) & (best[v[cand]] == r)
        wi = cand[win]
        if len(wi) == 0:
            break
        sel[wi] = True
        used[u[wi]] = True
        used[v[wi]] = True
    return sel


def _bn_fold(bnp, bias):
    g, b_, m, var = bnp[0], bnp[1], bnp[2], bnp[3]
    a = (g / np.sqrt(var + EPS)).astype(np.float32)
    c = (b_ - m * a).astype(np.float32)
    if bias is not None:
        c = (c + bias * a).astype(np.float32)
    return a, c


def kernel(x, edge_index, batch, pool_w, pool_b,
           c1_w0, c1_w1, c1_b, c2_w0, c2_w1, c2_b,
           bn1, bn2, bn3, bn4,
           lin1_w, lin1_b, lin2_w, lin2_b, lin3_w, lin3_b):
    LAST_EXEC_TIMES.clear()
    x = np.asarray(x, dtype=np.float32)
    u = np.asarray(edge_index[0], dtype=np.int64)
    v = np.asarray(edge_index[1], dtype=np.int64)
    batch = np.asarray(batch, dtype=np.int64)

    a1, bias1 = _bn_fold(np.asarray(bn1), np.asarray(c1_b))
    a2, bias2 = _bn_fold(np.asarray(bn2), np.asarray(c2_b))
    w01 = np.concatenate([np.asarray(c1_w0) * a1, np.asarray(c1_w1) * a1],
                         axis=1).astype(np.float32)
    w2cat = np.concatenate([np.asarray(c2_w0) * a2, np.asarray(c2_w1) * a2],
                           axis=1).astype(np.float32)
    w2aug = np.concatenate(
        [w2cat, np.concatenate([bias2, np.zeros(512, np.float32)])[None, :]],
        axis=0).astype(np.float32)  # [513, 1024]

    # ---- K12: Z = x @ w01 (node sharded) ----
    in_maps = []
    for c in range(NCORE):
        in_maps.append({
            "xT": np.ascontiguousarray(x[c * NPC:(c + 1) * NPC].T),
            "wcat": w01,
        })
    rz = _run("k12", in_maps)
    Z = np.concatenate([rz[c]["z"] for c in range(NCORE)], axis=0)

    # ---- host: scores, matching, coarsening ----
    pw = np.asarray(pool_w, dtype=np.float32)
    p1 = (x @ pw[:F]).ravel().astype(np.float32)
    p2 = (x @ pw[F:]).ravel().astype(np.float32)
    score = _scores(p1, p2, u, v)
    order = np.argsort(-score, kind="stable")
    sel = _greedy_match(u, v, order)

    rep = np.minimum(u, v)
    cluster = np.arange(N, dtype=np.int64)
    cluster[u[sel]] = rep[sel]
    cluster[v[sel]] = rep[sel]
    node_score = np.ones(N, dtype=np.float32)
    node_score[rep[sel]] = score[sel]
    valid = cluster == np.arange(N)
    vid = np.where(valid)[0]
    nv = len(vid)

    cu = cluster[u]
    cv = cluster[v]
    nonloop = cu != cv
    kept = np.unique(cu[nonloop] * N + cv[nonloop])
    cu_k = kept // N
    cv_k = kept % N
    deg = np.bincount(cu_k, minlength=N).astype(np.float32)
    dis = np.where(deg > 0, 1.0 / np.sqrt(np.maximum(deg, 1e-12)),
                   0.0).astype(np.float32)
    nrm = (-dis[cu_k] * dis[cv_k]).astype(np.float32)

    # ---- layout: graph-aligned padded positions ----
    gb = batch[vid]
    segs = np.bincount(gb, minlength=B)
    pos = np.empty(nv, dtype=np.int64)
    off = 0
    win_graph = []  # graph id of each 128-row window (global), -1 = padding
    for g in range(B):
        idx = np.where(gb == g)[0]
        pos[idx] = off + np.arange(len(idx))
        gpad = -(-len(idx) // 128) * 128
        win_graph += [g] * (gpad // 128)
        off += gpad
    NVP_tot = NCORE * NVP
    assert off <= NVP_tot, f"padded rows {off} > {NVP_tot}"
    win_graph += [-1] * ((NVP_tot - off) // 128 + (1 if off % 128 else 0))
    win_graph = np.array(win_graph[:NVP_tot // 128], dtype=np.int64)

    # per-row graph id (127 for padding)
    rowg = np.full(NVP_tot, 127, dtype=np.int64)
    rowg[pos] = gb

    # ---- host: combine Z -> t1 (with bias) and y1 (valid compact order) ----
    s_v = node_score[vid].astype(np.float32)[:, None]
    # partner of each valid rep (or -1)
    partner = np.full(N, -1, dtype=np.int64)
    ru, rv, rr = u[sel], v[sel], rep[sel]
    other = np.where(rr == ru, rv, ru)
    partner[rr] = other
    pvid = partner[vid]
    has_p = pvid >= 0
    t1v = Z[vid, :512] * s_v
    y1v = Z[vid, 512:] * s_v
    t1v[has_p] += s_v[has_p] * Z[pvid[has_p], :512]
    y1v[has_p] += s_v[has_p] * Z[pvid[has_p], 512:]
    t1v += bias1[None, :]

    # scatter into padded per-core node tensors
    def to_padded(av):
        out = np.zeros((NVP_tot, av.shape[1]), dtype=np.float32)
        out[pos] = av
        return out

    t1p = to_padded(t1v)
    onesp = np.zeros((NVP_tot,), dtype=np.float32)
    onesp[pos] = 1.0

    # ---- edges by destination core; per-window chunk packing ----
    dstp = pos[np.searchsorted(vid, cv_k)]  # vid is sorted; cv_k are valid ids
    srcv = np.searchsorted(vid, cu_k)       # src index in valid-compact order
    dcore = dstp // NVP

    esrc_a = np.zeros((NCORE, 128, NCH), dtype=np.int32)
    edst_a = np.full((NCORE, 128, NCH), 127.0, dtype=np.float32)
    enrm_a = np.zeros((NCORE, 128, NCH), dtype=np.float32)
    uniq_per_core = []
    for c in range(NCORE):
        em = np.where(dcore == c)[0]
        dl = dstp[em] - c * NVP
        o = np.argsort(dl, kind="stable")
        em = em[o]
        dl = dl[o]
        usrc, esrc_loc = np.unique(srcv[em], return_inverse=True)
        assert len(usrc) <= NU, f"core {c}: {len(usrc)} unique srcs > {NU}"
        uniq_per_core.append(usrc)
        w_of = dl // 128
        # slot assignment within each window
        for w in np.unique(w_of):
            ei = np.where(w_of == w)[0]
            assert len(ei) <= CPW * 128, \
                f"core {c} window {w}: {len(ei)} edges > {CPW * 128}"
            sl = np.arange(len(ei))
            chunk = w * CPW + sl // 128
            lane = sl % 128
            esrc_a[c, lane, chunk] = esrc_loc[ei]
            edst_a[c, lane, chunk] = (dl[ei] - w * 128).astype(np.float32)
            enrm_a[c, lane, chunk] = nrm[em[ei]]

    # ---- K3 ----
    in_maps = []
    for c in range(NCORE):
        y1c = np.zeros((NU, 512), dtype=ml_dtypes.bfloat16)
        y1c[:len(uniq_per_core[c])] = y1v[uniq_per_core[c]].astype(
            ml_dtypes.bfloat16)
        in_maps.append({
            "t1": t1p[c * NVP:(c + 1) * NVP],
            "y1c": y1c,
            "esrc": esrc_a[c], "edst": edst_a[c], "enrm": enrm_a[c],
            "w2": w2aug,
            "onesr": onesp[None, c * NVP:(c + 1) * NVP],
        })
    r3 = _run("k3", in_maps)
    zz2 = np.concatenate([r3[c]["zz2"] for c in range(NCORE)], axis=0)
    y2v = zz2[pos, 512:]  # valid-compact order

    # ---- K4 ----
    in_maps = []
    for c in range(NCORE):
        y2c = np.zeros((NU, 512), dtype=ml_dtypes.bfloat16)
        y2c[:len(uniq_per_core[c])] = y2v[uniq_per_core[c]].astype(
            ml_dtypes.bfloat16)
        bg = rowg[c * NVP:(c + 1) * NVP].reshape(NWIN, 128).T
        in_maps.append({
            "t2": zz2[c * NVP:(c + 1) * NVP, :512],
            "y2c": y2c,
            "esrc": esrc_a[c], "edst": edst_a[c], "enrm": enrm_a[c],
            "batchg": np.ascontiguousarray(bg.astype(np.float32)),
        })
    r4 = _run("k4", in_maps)

    # ---- host: pooling combine + MLP ----
    hsum = np.zeros((B, 512), dtype=np.float32)
    hmax = np.zeros((B, 512), dtype=np.float32)  # relu output >= 0
    for c in range(NCORE):
        hsum += r4[c]["hsum"]
        hm = r4[c]["hmaxt"]  # [128, 4*NWIN]
        for wloc in range(NWIN):
            g = win_graph[c * NWIN + wloc]
            if g < 0:
                continue
            vec = hm[:, wloc * 4:wloc * 4 + 4].T.reshape(512)
            hmax[g] = np.maximum(hmax[g], vec)
    cnt = np.bincount(gb, minlength=B).astype(np.float32)
    hmean = hsum / np.maximum(cnt, 1.0)[:, None]
    g = np.concatenate([hmax, hmean], axis=1)

    a3, c3 = _bn_fold(np.asarray(bn3), None)
    a4, c4 = _bn_fold(np.asarray(bn4), None)
    g = np.maximum((g @ np.asarray(lin1_w) + np.asarray(lin1_b)) * a3 + c3, 0)
    g = np.maximum((g @ np.asarray(lin2_w) + np.asarray(lin2_b)) * a4 + c4, 0)
    feature = g.astype(np.float32)
    zf = np.maximum(g @ np.asarray(lin3_w) + np.asarray(lin3_b), 0)
    zs = zf - zf.max(axis=-1, keepdims=True)
    out = (zs - np.log(np.exp(zs).sum(axis=-1, keepdims=True))).astype(
        np.float32)
    return out, feature


# revision 10
# speedup vs baseline: 1.0343x; 1.0343x over previous
"""Self-contained Trainium2 kernel for nn_Net_69183333204554 (GNN message
passing: EdgePooling -> 2x ChebConv+BN+ReLU -> global max/mean pool -> MLP).

Strategy (8 NeuronCores, SPMD):
  K12: Z = x @ [c1_w0*a1 | c1_w1*a1]  -- node-sharded dense matmul (fp32r PE),
       the dominant memory/compute pass over x (134 MB).
  host: edge scores (fp32, validated to reproduce the reference's greedy
       matching exactly), greedy maximal matching, cluster/coarsen/dedup,
       index-plumbing, routing of gathered rows between phases.
  K3:  ChebConv1 message passing (indirect row gathers + indicator-matmul
       segment-sum on PE) + ReLU + ChebConv2 dense matmuls.
  K4:  ChebConv2 message passing + ReLU + global max/sum pooling on device.
  host: tiny readout MLP ([8,1024] -> [8,4]/[8,256]).

All shapes hardcoded for the fixed problem instance (N=32768, E=131072, B=8).
"""
import numpy as np
import ml_dtypes
from contextlib import ExitStack

try:  # persistent XLA/NEFF compile cache across processes (best-effort)
    import jax as _jax
    _jax.config.update("jax_compilation_cache_dir", "/tmp/.jax_bass_cache")
    _jax.config.update("jax_persistent_cache_min_entry_size_bytes", -1)
    _jax.config.update("jax_persistent_cache_min_compile_time_secs", 0.0)
except Exception:
    pass

import concourse.bass as bass
import concourse.tile as tile
from concourse import bacc, mybir
from concourse.bass_utils import run_bass_kernel_spmd
from concourse.masks import make_identity

F32 = mybir.dt.float32
F32R = mybir.dt.float32r
BF16 = mybir.dt.bfloat16
I32 = mybir.dt.int32

NCORE = 8
N = 32768
E = 131072
B = 8
F = 1025
NPC = N // NCORE          # 4096 nodes per core in K12
KCH = 8                   # full 128-rows K chunks in F (1025 = 8*128 + 1)
NVP = 2304                # padded valid rows per core (18 tiles of 128)
NWIN = NVP // 128         # 18 windows / node tiles per core
CPW = 8                   # max edge chunks (of 128) per 128-dst window
NCH = NWIN * CPW          # 144 chunk slots per core
NIW = CPW * 128           # 1024 gather indices per window
I16 = mybir.dt.int16
NU = 10496                # padded unique-source rows per core
EPS = np.float32(1e-5)
NEG = np.float32(-1e30)

# exec times of the last kernel() call, for test harnesses: list of
# (name, exec_time_ns or None)
LAST_EXEC_TIMES = []

_PROGS = {}


def _new_nc():
    return bacc.Bacc("TRN2", target_bir_lowering=False, debug=False,
                     num_devices=NCORE)


# --------------------------------------------------------------------------
# K12: Z[4096, 1024] = xT.T @ wcat      (per core)
# --------------------------------------------------------------------------
def _build_k12():
    nc = _new_nc()
    xT = nc.dram_tensor("xT", [F, NPC], F32R, kind="ExternalInput").ap()
    wcat = nc.dram_tensor("wcat", [F, 1024], F32R, kind="ExternalInput").ap()
    z = nc.dram_tensor("z", [NPC, 1024], F32, kind="ExternalOutput").ap()

    with tile.TileContext(nc) as tc, ExitStack() as ctx:
        wpool = ctx.enter_context(tc.tile_pool(name="w", bufs=1))
        xpool = ctx.enter_context(tc.tile_pool(name="x", bufs=2))
        zpool = ctx.enter_context(tc.tile_pool(name="z", bufs=4))
        psum = ctx.enter_context(tc.tile_pool(name="ps", bufs=2, space="PSUM"))

        w_sb = []
        for k in range(KCH + 1):
            p = 128 if k < KCH else 1
            t = wpool.tile([p, 1024], F32R, tag=f"w{k}")
            nc.sync.dma_start(t[:], wcat[k * 128:k * 128 + p, :])
            w_sb.append(t)

        NB = NPC // 512  # 8 node blocks of 512
        for jb in range(NB):
            xb = []
            for k in range(KCH + 1):
                p = 128 if k < KCH else 1
                t = xpool.tile([p, 512], F32R, tag=f"x{k}")
                nc.sync.dma_start(t[:], xT[k * 128:k * 128 + p,
                                            jb * 512:(jb + 1) * 512])
                xb.append(t)
            for j4 in range(4):
                js = slice(j4 * 128, (j4 + 1) * 128)
                pts = []
                for h in range(2):
                    pt = psum.tile([128, 512], F32, tag=f"acc{h}")
                    pts.append(pt)
                for k in range(KCH + 1):
                    for h in range(2):
                        nc.tensor.matmul(
                            pts[h][:],
                            lhsT=xb[k][:, js],
                            rhs=w_sb[k][:, h * 512:(h + 1) * 512],
                            start=(k == 0),
                            stop=(k == KCH),
                        )
                for h in range(2):
                    zt = zpool.tile([128, 512], F32, tag="zt")
                    nc.vector.tensor_copy(zt[:], pts[h][:])
                    nc.sync.dma_start(
                        z[jb * 512 + j4 * 128:jb * 512 + (j4 + 1) * 128,
                          h * 512:(h + 1) * 512],
                        zt[:])
    nc.compile()
    return nc


# --------------------------------------------------------------------------
# K3: message passing 1 + relu + conv2 matmuls  (per core)
# --------------------------------------------------------------------------
def _build_k3():
    nc = _new_nc()
    t1 = nc.dram_tensor("t1", [NVP, 512], F32, kind="ExternalInput").ap()
    y1c = nc.dram_tensor("y1c", [NU, 512], BF16, kind="ExternalInput").ap()
    esrc = nc.dram_tensor("esrc", [128, NWIN * NIW // 16], I16,
                          kind="ExternalInput").ap()
    edst = nc.dram_tensor("edst", [128, NCH], F32, kind="ExternalInput").ap()
    enrm = nc.dram_tensor("enrm", [128, NCH], F32, kind="ExternalInput").ap()
    w2 = nc.dram_tensor("w2", [513, 1024], F32R, kind="ExternalInput").ap()
    onesr = nc.dram_tensor("onesr", [1, NVP], F32R, kind="ExternalInput").ap()
    zz2 = nc.dram_tensor("zz2", [NVP, 1024], F32, kind="ExternalOutput").ap()

    with tile.TileContext(nc) as tc, ExitStack() as ctx:
        const = ctx.enter_context(tc.tile_pool(name="const", bufs=1))
        gat = ctx.enter_context(tc.tile_pool(name="gat", bufs=6))
        hseq = ctx.enter_context(tc.tile_pool(name="hseq", bufs=3))
        opool = ctx.enter_context(tc.tile_pool(name="o", bufs=4))
        psum = ctx.enter_context(tc.tile_pool(name="ps", bufs=2, space="PSUM"))
        psum2 = ctx.enter_context(tc.tile_pool(name="ps2", bufs=4, space="PSUM"))

        iota_i = const.tile([128, 128], I32)
        nc.gpsimd.iota(iota_i[:], pattern=[[1, 128]], base=0,
                       channel_multiplier=0)
        iota_f = const.tile([128, 128], F32)
        nc.vector.tensor_copy(iota_f[:], iota_i[:])
        ident = const.tile([128, 128], F32)
        make_identity(nc, ident[:])
        identr = const.tile([128, 128], F32R)
        nc.vector.tensor_copy(identr[:], ident[:])

        esrc_sb = const.tile([128, NWIN * NIW // 16], I16)
        nc.sync.dma_start(esrc_sb[:], esrc[:, :])
        edst_sb = const.tile([128, NCH], F32)
        nc.sync.dma_start(edst_sb[:], edst[:, :])
        enrm_sb = const.tile([128, NCH], F32)
        nc.sync.dma_start(enrm_sb[:], enrm[:, :])

        t1_sb = const.tile([128, NWIN * 512], F32)
        nc.sync.dma_start(
            t1_sb[:].rearrange("p (t f) -> p t f", f=512),
            t1.rearrange("(t p) f -> p t f", p=128))

        onesr_sb = const.tile([1, NVP], F32R)
        nc.sync.dma_start(onesr_sb[:], onesr[:, :])

        w2_sb = []
        for k in range(5):
            p = 128 if k < 4 else 1
            t = const.tile([p, 1024], F32R, tag=f"w2{k}")
            nc.sync.dma_start(t[:], w2[k * 128:k * 128 + p, :])
            w2_sb.append(t)

        h1T = const.tile([128, 4 * NVP], F32R)

        for w in range(NWIN):
            pt = psum.tile([128, 512], F32, tag="tx")
            y_t = gat.tile([128, NIW // 128 * 512], BF16, tag="y")
            nc.gpsimd.dma_gather(
                out_ap=y_t[:].rearrange("p (c f) -> p c f", f=512),
                in_ap=y1c[:, :],
                idxs_ap=esrc_sb[:, w * (NIW // 16):(w + 1) * (NIW // 16)],
                num_idxs=NIW, num_idxs_reg=NIW, elem_size=512)
            for cch in range(CPW):
                s = w * CPW + cch
                ind = gat.tile([128, 128], BF16, tag="ind")
                nc.vector.tensor_scalar(
                    out=ind[:], in0=iota_f[:],
                    scalar1=edst_sb[:, s:s + 1],
                    scalar2=enrm_sb[:, s:s + 1],
                    op0=mybir.AluOpType.is_equal,
                    op1=mybir.AluOpType.mult)
                nc.tensor.matmul(pt[:], lhsT=ind[:],
                                 rhs=y_t[:, cch * 512:(cch + 1) * 512],
                                 start=(cch == 0), stop=(cch == CPW - 1))
            h1_t = hseq.tile([128, 512], F32R, tag="h1")
            nc.vector.tensor_add(h1_t[:], pt[:],
                                 t1_sb[:, w * 512:(w + 1) * 512])
            nc.vector.tensor_scalar_max(h1_t[:], h1_t[:], 0.0)
            for k4 in range(4):
                ptr = psum2.tile([128, 128], F32R, tag="tr")
                nc.tensor.transpose(
                    out=ptr[:],
                    in_=h1_t[:, k4 * 128:(k4 + 1) * 128],
                    identity=identr[:])
                nc.vector.tensor_copy(
                    h1T[:, k4 * NVP + w * 128:k4 * NVP + (w + 1) * 128],
                    ptr[:])

        for j in range(NWIN):
            for h in range(2):
                hs = slice(h * 512, (h + 1) * 512)
                po = psum.tile([128, 512], F32, tag="o")
                for k4 in range(4):
                    nc.tensor.matmul(
                        po[:],
                        lhsT=h1T[:, k4 * NVP + j * 128:
                                 k4 * NVP + (j + 1) * 128],
                        rhs=w2_sb[k4][:, hs],
                        start=(k4 == 0), stop=False)
                nc.tensor.matmul(
                    po[:],
                    lhsT=onesr_sb[:, j * 128:(j + 1) * 128],
                    rhs=w2_sb[4][:, hs],
                    start=False, stop=True)
                ot = opool.tile([128, 512], F32, tag="ot")
                nc.vector.tensor_copy(ot[:], po[:])
                nc.sync.dma_start(zz2[j * 128:(j + 1) * 128, hs], ot[:])
    nc.compile()
    return nc


# --------------------------------------------------------------------------
# K4: message passing 2 + relu + pooling  (per core)
# --------------------------------------------------------------------------
def _build_k4():
    nc = _new_nc()
    t2 = nc.dram_tensor("t2", [NVP, 512], F32, kind="ExternalInput").ap()
    y2c = nc.dram_tensor("y2c", [NU, 512], BF16, kind="ExternalInput").ap()
    esrc = nc.dram_tensor("esrc", [128, NWIN * NIW // 16], I16,
                          kind="ExternalInput").ap()
    edst = nc.dram_tensor("edst", [128, NCH], F32, kind="ExternalInput").ap()
    enrm = nc.dram_tensor("enrm", [128, NCH], F32, kind="ExternalInput").ap()
    batchg = nc.dram_tensor("batchg", [128, NWIN], F32,
                            kind="ExternalInput").ap()
    hsum = nc.dram_tensor("hsum", [8, 512], F32, kind="ExternalOutput").ap()
    hmaxt = nc.dram_tensor("hmaxt", [128, 4 * NWIN], F32,
                           kind="ExternalOutput").ap()

    with tile.TileContext(nc) as tc, ExitStack() as ctx:
        const = ctx.enter_context(tc.tile_pool(name="const", bufs=1))
        gat = ctx.enter_context(tc.tile_pool(name="gat", bufs=6))
        hseq = ctx.enter_context(tc.tile_pool(name="hseq", bufs=3))
        psum = ctx.enter_context(tc.tile_pool(name="ps", bufs=2, space="PSUM"))
        psum2 = ctx.enter_context(tc.tile_pool(name="ps2", bufs=4, space="PSUM"))
        psmax = ctx.enter_context(tc.tile_pool(name="psm", bufs=1, space="PSUM"))

        iota_i = const.tile([128, 128], I32)
        nc.gpsimd.iota(iota_i[:], pattern=[[1, 128]], base=0,
                       channel_multiplier=0)
        iota_f = const.tile([128, 128], F32)
        nc.vector.tensor_copy(iota_f[:], iota_i[:])
        iota8_f = const.tile([128, 8], F32)
        nc.vector.tensor_copy(iota8_f[:], iota_i[:, :8])
        ident = const.tile([128, 128], F32)
        make_identity(nc, ident[:])
        identr = const.tile([128, 128], F32R)
        nc.vector.tensor_copy(identr[:], ident[:])

        esrc_sb = const.tile([128, NWIN * NIW // 16], I16)
        nc.sync.dma_start(esrc_sb[:], esrc[:, :])
        edst_sb = const.tile([128, NCH], F32)
        nc.sync.dma_start(edst_sb[:], edst[:, :])
        enrm_sb = const.tile([128, NCH], F32)
        nc.sync.dma_start(enrm_sb[:], enrm[:, :])
        batchg_sb = const.tile([128, NWIN], F32)
        nc.sync.dma_start(batchg_sb[:], batchg[:, :])

        t2_sb = const.tile([128, NWIN * 512], F32)
        nc.sync.dma_start(
            t2_sb[:].rearrange("p (t f) -> p t f", f=512),
            t2.rearrange("(t p) f -> p t f", p=128))

        hmax_sb = const.tile([128, 4 * NWIN], F32)
        ps_hs = psmax.tile([8, 512], F32)

        for w in range(NWIN):
            pt = psum.tile([128, 512], F32, tag="tx")
            y_t = gat.tile([128, NIW // 128 * 512], BF16, tag="y")
            nc.gpsimd.dma_gather(
                out_ap=y_t[:].rearrange("p (c f) -> p c f", f=512),
                in_ap=y2c[:, :],
                idxs_ap=esrc_sb[:, w * (NIW // 16):(w + 1) * (NIW // 16)],
                num_idxs=NIW, num_idxs_reg=NIW, elem_size=512)
            for cch in range(CPW):
                s = w * CPW + cch
                ind = gat.tile([128, 128], BF16, tag="ind")
                nc.vector.tensor_scalar(
                    out=ind[:], in0=iota_f[:],
                    scalar1=edst_sb[:, s:s + 1],
                    scalar2=enrm_sb[:, s:s + 1],
                    op0=mybir.AluOpType.is_equal,
                    op1=mybir.AluOpType.mult)
                nc.tensor.matmul(pt[:], lhsT=ind[:],
                                 rhs=y_t[:, cch * 512:(cch + 1) * 512],
                                 start=(cch == 0), stop=(cch == CPW - 1))
            h2_t = hseq.tile([128, 512], F32R, tag="h2")
            nc.vector.tensor_add(h2_t[:], pt[:],
                                 t2_sb[:, w * 512:(w + 1) * 512])
            nc.vector.tensor_scalar_max(h2_t[:], h2_t[:], 0.0)

            # per-graph sums via batch-indicator matmul (accumulate over all
            # windows into one PSUM tile)
            ind_b = gat.tile([128, 8], F32R, tag="indb")
            nc.vector.tensor_scalar(
                out=ind_b[:], in0=iota8_f[:],
                scalar1=batchg_sb[:, w:w + 1], scalar2=None,
                op0=mybir.AluOpType.is_equal)
            nc.tensor.matmul(ps_hs[:], lhsT=ind_b[:], rhs=h2_t[:],
                             start=(w == 0), stop=(w == NWIN - 1))

            # per-tile channel max via transpose + free-dim reduce
            for k4 in range(4):
                ptr = psum2.tile([128, 128], F32R, tag="tr")
                nc.tensor.transpose(
                    out=ptr[:],
                    in_=h2_t[:, k4 * 128:(k4 + 1) * 128],
                    identity=identr[:])
                nc.vector.tensor_reduce(
                    out=hmax_sb[:, w * 4 + k4:w * 4 + k4 + 1],
                    in_=ptr[:], axis=mybir.AxisListType.X,
                    op=mybir.AluOpType.max)

        hs_sb = const.tile([8, 512], F32)
        nc.vector.tensor_copy(hs_sb[:], ps_hs[:])
        nc.sync.dma_start(hsum[:, :], hs_sb[:])
        nc.sync.dma_start(hmaxt[:, :], hmax_sb[:])
    nc.compile()
    return nc


def _get_prog(name):
    if name not in _PROGS:
        _PROGS[name] = {"k12": _build_k12, "k3": _build_k3,
                        "k4": _build_k4}[name]()
    return _PROGS[name]


def _run(name, in_maps):
    nc = _get_prog(name)
    r = run_bass_kernel_spmd(nc, in_maps, core_ids=list(range(NCORE)))
    LAST_EXEC_TIMES.append((name, r.exec_time_ns))
    return r.results


# --------------------------------------------------------------------------
# host-side pipeline pieces
# --------------------------------------------------------------------------
def _scores(p1, p2, u, v):
    e = p1[u] + p2[v]
    m = np.full(N, -np.inf, dtype=np.float32)
    np.maximum.at(m, v, e)
    ex = np.exp(e - m[v])
    ssum = np.zeros(N, dtype=np.float32)
    np.add.at(ssum, v, ex)
    return (ex / ssum[v] + np.float32(0.5)).astype(np.float32)


def _greedy_match(u, v, order):
    """Sequential greedy matching, replicated via rounds of local-max
    matching (equivalent under the strict total order given by `order`)."""
    rank = np.empty(E, dtype=np.int64)
    rank[order] = np.arange(E)
    used = np.zeros(N, dtype=bool)
    sel = np.zeros(E, dtype=bool)
    INF = np.iinfo(np.int64).max
    cand = np.arange(E)
    while True:
        cand = cand[~(used[u[cand]] | used[v[cand]])]
        if len(cand) == 0:
            break
        r = rank[cand]
        best = np.full(N, INF, dtype=np.int64)
        np.minimum.at(best, u[cand], r)
        np.minimum.at(best, v[cand], r)
        win = (best[u[cand]] == r) & (best[v[cand]] == r)
        wi = cand[win]
        if len(wi) == 0:
            break
        sel[wi] = True
        used[u[wi]] = True
        used[v[wi]] = True
    return sel


def _bn_fold(bnp, bias):
    g, b_, m, var = bnp[0], bnp[1], bnp[2], bnp[3]
    a = (g / np.sqrt(var + EPS)).astype(np.float32)
    c = (b_ - m * a).astype(np.float32)
    if bias is not None:
        c = (c + bias * a).astype(np.float32)
    return a, c


def kernel(x, edge_index, batch, pool_w, pool_b,
           c1_w0, c1_w1, c1_b, c2_w0, c2_w1, c2_b,
           bn1, bn2, bn3, bn4,
           lin1_w, lin1_b, lin2_w, lin2_b, lin3_w, lin3_b):
    LAST_EXEC_TIMES.clear()
    x = np.asarray(x, dtype=np.float32)
    u = np.asarray(edge_index[0], dtype=np.int64)
    v = np.asarray(edge_index[1], dtype=np.int64)
    batch = np.asarray(batch, dtype=np.int64)

    a1, bias1 = _bn_fold(np.asarray(bn1), np.asarray(c1_b))
    a2, bias2 = _bn_fold(np.asarray(bn2), np.asarray(c2_b))
    w01 = np.concatenate([np.asarray(c1_w0) * a1, np.asarray(c1_w1) * a1],
                         axis=1).astype(np.float32)
    w2cat = np.concatenate([np.asarray(c2_w0) * a2, np.asarray(c2_w1) * a2],
                           axis=1).astype(np.float32)
    w2aug = np.concatenate(
        [w2cat, np.concatenate([bias2, np.zeros(512, np.float32)])[None, :]],
        axis=0).astype(np.float32)  # [513, 1024]

    # ---- K12: Z = x @ w01 (node sharded) ----
    in_maps = []
    for c in range(NCORE):
        in_maps.append({
            "xT": np.ascontiguousarray(x[c * NPC:(c + 1) * NPC].T),
            "wcat": w01,
        })
    rz = _run("k12", in_maps)
    Z = np.concatenate([rz[c]["z"] for c in range(NCORE)], axis=0)

    # ---- host: scores, matching, coarsening ----
    pw = np.asarray(pool_w, dtype=np.float32)
    p1 = (x @ pw[:F]).ravel().astype(np.float32)
    p2 = (x @ pw[F:]).ravel().astype(np.float32)
    score = _scores(p1, p2, u, v)
    order = np.argsort(-score, kind="stable")
    sel = _greedy_match(u, v, order)

    rep = np.minimum(u, v)
    cluster = np.arange(N, dtype=np.int64)
    cluster[u[sel]] = rep[sel]
    cluster[v[sel]] = rep[sel]
    node_score = np.ones(N, dtype=np.float32)
    node_score[rep[sel]] = score[sel]
    valid = cluster == np.arange(N)
    vid = np.where(valid)[0]
    nv = len(vid)

    cu = cluster[u]
    cv = cluster[v]
    nonloop = cu != cv
    kept = np.unique(cu[nonloop] * N + cv[nonloop])
    cu_k = kept // N
    cv_k = kept % N
    deg = np.bincount(cu_k, minlength=N).astype(np.float32)
    dis = np.where(deg > 0, 1.0 / np.sqrt(np.maximum(deg, 1e-12)),
                   0.0).astype(np.float32)
    nrm = (-dis[cu_k] * dis[cv_k]).astype(np.float32)

    # ---- layout: graph-aligned padded positions ----
    gb = batch[vid]
    segs = np.bincount(gb, minlength=B)
    pos = np.empty(nv, dtype=np.int64)
    off = 0
    win_graph = []  # graph id of each 128-row window (global), -1 = padding
    for g in range(B):
        idx = np.where(gb == g)[0]
        pos[idx] = off + np.arange(len(idx))
        gpad = -(-len(idx) // 128) * 128
        win_graph += [g] * (gpad // 128)
        off += gpad
    NVP_tot = NCORE * NVP
    assert off <= NVP_tot, f"padded rows {off} > {NVP_tot}"
    win_graph += [-1] * ((NVP_tot - off) // 128 + (1 if off % 128 else 0))
    win_graph = np.array(win_graph[:NVP_tot // 128], dtype=np.int64)

    # per-row graph id (127 for padding)
    rowg = np.full(NVP_tot, 127, dtype=np.int64)
    rowg[pos] = gb

    # ---- host: combine Z -> t1 (with bias) and y1 (valid compact order) ----
    s_v = node_score[vid].astype(np.float32)[:, None]
    # partner of each valid rep (or -1)
    partner = np.full(N, -1, dtype=np.int64)
    ru, rv, rr = u[sel], v[sel], rep[sel]
    other = np.where(rr == ru, rv, ru)
    partner[rr] = other
    pvid = partner[vid]
    has_p = pvid >= 0
    t1v = Z[vid, :512] * s_v
    y1v = Z[vid, 512:] * s_v
    t1v[has_p] += s_v[has_p] * Z[pvid[has_p], :512]
    y1v[has_p] += s_v[has_p] * Z[pvid[has_p], 512:]
    t1v += bias1[None, :]

    # scatter into padded per-core node tensors
    def to_padded(av):
        out = np.zeros((NVP_tot, av.shape[1]), dtype=np.float32)
        out[pos] = av
        return out

    t1p = to_padded(t1v)
    onesp = np.zeros((NVP_tot,), dtype=np.float32)
    onesp[pos] = 1.0

    # ---- edges by destination core; per-window chunk packing ----
    dstp = pos[np.searchsorted(vid, cv_k)]  # vid is sorted; cv_k are valid ids
    srcv = np.searchsorted(vid, cu_k)       # src index in valid-compact order
    dcore = dstp // NVP

    esrc_a = np.zeros((NCORE, 128, NWIN * NIW // 16), dtype=np.int16)
    edst_a = np.full((NCORE, 128, NCH), 127.0, dtype=np.float32)
    enrm_a = np.zeros((NCORE, 128, NCH), dtype=np.float32)
    uniq_per_core = []
    for c in range(NCORE):
        em = np.where(dcore == c)[0]
        dl = dstp[em] - c * NVP
        o = np.argsort(dl, kind="stable")
        em = em[o]
        dl = dl[o]
        usrc, esrc_loc = np.unique(srcv[em], return_inverse=True)
        assert len(usrc) <= NU, f"core {c}: {len(usrc)} unique srcs > {NU}"
        uniq_per_core.append(usrc)
        w_of = dl // 128
        for w in np.unique(w_of):
            ei = np.where(w_of == w)[0]
            assert len(ei) <= NIW, \
                f"core {c} window {w}: {len(ei)} edges > {NIW}"
            sl = np.arange(len(ei))
            chunk = w * CPW + sl // 128
            lane = sl % 128
            # int16 gather indices, wrapped in 16 partitions, replicated
            # across the 8 partition groups
            col = w * (NIW // 16) + sl // 16
            vals = esrc_loc[ei].astype(np.int16)
            for g in range(8):
                esrc_a[c, 16 * g + sl % 16, col] = vals
            edst_a[c, lane, chunk] = (dl[ei] - w * 128).astype(np.float32)
            enrm_a[c, lane, chunk] = nrm[em[ei]]

    # ---- K3 ----
    in_maps = []
    for c in range(NCORE):
        y1c = np.zeros((NU, 512), dtype=ml_dtypes.bfloat16)
        y1c[:len(uniq_per_core[c])] = y1v[uniq_per_core[c]].astype(
            ml_dtypes.bfloat16)
        in_maps.append({
            "t1": t1p[c * NVP:(c + 1) * NVP],
            "y1c": y1c,
            "esrc": esrc_a[c], "edst": edst_a[c], "enrm": enrm_a[c],
            "w2": w2aug,
            "onesr": onesp[None, c * NVP:(c + 1) * NVP],
        })
    r3 = _run("k3", in_maps)
    zz2 = np.concatenate([r3[c]["zz2"] for c in range(NCORE)], axis=0)
    y2v = zz2[pos, 512:]  # valid-compact order

    # ---- K4 ----
    in_maps = []
    for c in range(NCORE):
        y2c = np.zeros((NU, 512), dtype=ml_dtypes.bfloat16)
        y2c[:len(uniq_per_core[c])] = y2v[uniq_per_core[c]].astype(
            ml_dtypes.bfloat16)
        bg = rowg[c * NVP:(c + 1) * NVP].reshape(NWIN, 128).T
        in_maps.append({
            "t2": zz2[c * NVP:(c + 1) * NVP, :512],
            "y2c": y2c,
            "esrc": esrc_a[c], "edst": edst_a[c], "enrm": enrm_a[c],
            "batchg": np.ascontiguousarray(bg.astype(np.float32)),
        })
    r4 = _run("k4", in_maps)

    # ---- host: pooling combine + MLP ----
    hsum = np.zeros((B, 512), dtype=np.float32)
    hmax = np.zeros((B, 512), dtype=np.float32)  # relu output >= 0
    for c in range(NCORE):
        hsum += r4[c]["hsum"]
        hm = r4[c]["hmaxt"]  # [128, 4*NWIN]
        for wloc in range(NWIN):
            g = win_graph[c * NWIN + wloc]
            if g < 0:
                continue
            vec = hm[:, wloc * 4:wloc * 4 + 4].T.reshape(512)
            hmax[g] = np.maximum(hmax[g], vec)
    cnt = np.bincount(gb, minlength=B).astype(np.float32)
    hmean = hsum / np.maximum(cnt, 1.0)[:, None]
    g = np.concatenate([hmax, hmean], axis=1)

    a3, c3 = _bn_fold(np.asarray(bn3), None)
    a4, c4 = _bn_fold(np.asarray(bn4), None)
    g = np.maximum((g @ np.asarray(lin1_w) + np.asarray(lin1_b)) * a3 + c3, 0)
    g = np.maximum((g @ np.asarray(lin2_w) + np.asarray(lin2_b)) * a4 + c4, 0)
    feature = g.astype(np.float32)
    zf = np.maximum(g @ np.asarray(lin3_w) + np.asarray(lin3_b), 0)
    zs = zf - zf.max(axis=-1, keepdims=True)
    out = (zs - np.log(np.exp(zs).sum(axis=-1, keepdims=True))).astype(
        np.float32)
    return out, feature
